# revision 14
# baseline (speedup 1.0000x reference)
"""CNF vector-field + exact Jacobian-trace kernel for Trainium2 (8 NeuronCores).

Math: for each sample x (D=32), with inp = [x, t] (33,):
  h1 = tanh(inp @ W1 + b1); h2 = tanh(h1 @ W2 + b2); dx = h2 @ W3 + b3
  div = trace(J) = d1^T C d2,  C = W2 * (W3 @ W1r)^T  (elementwise),
  d_i = 1 - h_i^2,  W1r = W1[:32]
  out = [dx, div]  (B, 33)

v5 implementation notes:
  - all layout work on HOST: x^T, W3 chunk-packed, biases folded
    (bias1 = t*W1[32]+b1), and C itself precomputed on host (weight-only)
    so the device runs no mp/cmat chain at all
  - full bf16 datapath (PSUM accumulate stays f32), rel err ~5e-3 vs
    the 2e-2 gate
  - d1 = 1 - h1^2 (GpSimd square + DVE affine); h2sq split GpSimd/DVE;
    b3 via ACT Identity copy-bias; div via (-1)-column matmul over
    E = (h2sq-1)*gt
  - PSUM: one open accumulation group per bank (hw requirement):
    z tag 4 banks (z1 cycle into z2), gt tag 2 banks (m-outer), small 2
  - two HWDGE rings, 2-chunk splits: sync = hot,w2a,w2b + out;
    scalar = big0,cma,cmb,w3pn. z2 runs k-pair super-rounds as W2
    halves land; ACT table preload emitted after the scalar issues
  - PE warmup spinner ramps the DVFS clock during the DMA-wait window
"""
import sys

for _p in ("/opt/trn_rl_repo", "/root/.axon_site/_ro/trn_rl_repo"):
    if _p not in sys.path:
        sys.path.append(_p)

import numpy as np
import ml_dtypes

BF16 = ml_dtypes.bfloat16
B, D, H = 2048, 32, 512
NCORES = 8
BC = B // NCORES          # 256 rows per core
NK = H // 128             # 4 chunks of the hidden dim
WARMUP = 12               # PE clock-ramp spinner matmuls

_CACHE = {}


def _build():
    import concourse.bass as bass
    import concourse.tile as tile
    from concourse import bacc, mybir

    f32 = mybir.dt.float32
    bf = mybir.dt.bfloat16
    AF = mybir.ActivationFunctionType
    ALU = mybir.AluOpType

    nc = bacc.Bacc("TRN2", target_bir_lowering=False, debug=False,
                   num_devices=NCORES)

    # big0 cols: [0:256]=x^T slice, [256:768]=W1r
    big0_ext = nc.dram_tensor("big0", [D, BC + H], bf,
                              kind="ExternalInput").ap()
    # chunk-packed on host: w2p[p, k*512+j] = W2[k*128+p, j]
    w2_ext = nc.dram_tensor("w2", [128, NK * H], bf, kind="ExternalInput").ap()
    cm_ext = nc.dram_tensor("cm", [128, NK * H], bf, kind="ExternalInput").ap()
    # w3pn cols: [0:128]=W3 chunk-packed (lhsT for dx), 128=-1
    w3pn_ext = nc.dram_tensor("w3pn", [128, 129], bf,
                              kind="ExternalInput").ap()
    # hot cols: [0:4]=b2 col-major, [4:8]=bias1 col-major, 8=b3 (rows 0:32)
    hot_ext = nc.dram_tensor("hot", [128, 9], f32, kind="ExternalInput").ap()
    out_ext = nc.dram_tensor("out", [D + 1, BC], f32, kind="ExternalOutput").ap()

    with tile.TileContext(nc) as tc:
        with tc.tile_pool(name="const", bufs=1) as cpool, \
             tc.tile_pool(name="work", bufs=1) as wpool, \
             tc.tile_pool(name="ps", bufs=1, space="PSUM") as pps:

            # One open accumulation group per PSUM bank (hw requirement).
            def zps(nm):
                return pps.tile([128, BC], f32, name=nm, tag="z", bufs=4)

            def gps(nm):
                return pps.tile([128, BC], f32, name=nm, tag="gt", bufs=2)

            def small_ps(nm, shape):
                return pps.tile(shape, f32, name=nm, tag="small", bufs=2)

            # -------- PE warmup spinner (ramps clock during DMA wait) -----
            wt = wpool.tile([1, BC], bf, name="wt")
            nc.gpsimd.memset(wt[:, :], 0.0)
            warm = small_ps("warm", [1, BC])
            for _ in range(WARMUP):
                nc.tensor.matmul(warm[:, :], wt[:, 0:1], wt[:, :],
                                 start=True, stop=True)

            # ------------- input DMAs (two rings, need-ordered) -----------
            hot = cpool.tile([128, 9], f32, name="hot")
            nc.sync.dma_start(out=hot[:, :], in_=hot_ext[:, :])

            big0 = cpool.tile([D, BC + H], bf, name="big0")
            nc.scalar.dma_start(out=big0[:, :], in_=big0_ext[:, :])
            xts = big0[:, 0:BC]
            w1p = big0[:, BC:BC + H]

            w2all = cpool.tile([128, NK * H], bf, name="w2all")
            for half in range(2):
                nc.sync.dma_start(
                    out=w2all[:, half * 2 * H:(half + 1) * 2 * H],
                    in_=w2_ext[:, half * 2 * H:(half + 1) * 2 * H])
            w2k = [w2all[:, k * H:(k + 1) * H] for k in range(NK)]

            cmat = cpool.tile([128, NK * H], bf, name="cmat")
            for half in range(2):
                nc.scalar.dma_start(
                    out=cmat[:, half * 2 * H:(half + 1) * 2 * H],
                    in_=cm_ext[:, half * 2 * H:(half + 1) * 2 * H])
            cmk = [cmat[:, k * H:(k + 1) * H] for k in range(NK)]

            w3pn = cpool.tile([128, 129], bf, name="w3pn")
            nc.scalar.dma_start(out=w3pn[:, :], in_=w3pn_ext[:, :])
            w3p = [w3pn[:, k * D:(k + 1) * D] for k in range(NK)]
            neg_col = w3pn[:, 128:129]

            # -------- ACT spline-table preload (after scalar issues) ------
            dm0 = wpool.tile([1, 1], f32, name="dm0")
            dm1 = wpool.tile([1, 1], f32, name="dm1")
            nc.gpsimd.memset(dm0[:, :], 0.0)
            nc.scalar.activation(dm1[:, :], dm0[:, :], AF.Tanh)

            # ---------------- layer 1 matmuls, then all tanh --------------
            z1s = []
            for m in range(NK):
                z1 = zps("z1")
                nc.tensor.matmul(z1[:, :], w1p[:, m * 128:(m + 1) * 128],
                                 xts[:, :], start=True, stop=True)
                z1s.append(z1)
            h1t = []
            for m in range(NK):
                h = wpool.tile([128, BC], bf, name=f"h1t_{m}")
                nc.scalar.activation(h[:, :], z1s[m][:, :], AF.Tanh,
                                     bias=hot[:, 4 + m:5 + m])
                h1t.append(h)

            # ---------------- layer 2: k-pair super-rounds ----------------
            z2s = [zps("z2") for _ in range(NK)]
            for k in range(NK):
                for m in range(NK):
                    nc.tensor.matmul(z2s[m][:, :],
                                     w2k[k][:, m * 128:(m + 1) * 128],
                                     h1t[k][:, :],
                                     start=(k == 0), stop=(k == NK - 1))

            # ------- d1 = 1 - h1^2 (GpSimd sq, DVE affine) ----------------
            d1t = []
            for m in range(NK):
                sq = wpool.tile([128, BC], bf, name=f"sq_{m}")
                nc.gpsimd.tensor_tensor(out=sq[:, :], in0=h1t[m][:, :],
                                        in1=h1t[m][:, :], op=ALU.mult)
                d1 = wpool.tile([128, BC], bf, name=f"d1_{m}")
                nc.vector.tensor_scalar(out=d1[:, :], in0=sq[:, :],
                                        scalar1=-1.0, scalar2=1.0,
                                        op0=ALU.mult, op1=ALU.add)
                d1t.append(d1)

            h2t = []
            for m in range(NK):
                h = wpool.tile([128, BC], bf, name=f"h2t_{m}")
                nc.scalar.activation(h[:, :], z2s[m][:, :], AF.Tanh,
                                     bias=hot[:, m:1 + m])
                h2t.append(h)

            # -------- h2sq split across GpSimd and DVE --------------------
            h2sq = []
            for m in range(NK):
                s2 = wpool.tile([128, BC], bf, name=f"h2sq_{m}")
                eng = nc.gpsimd if m < 2 else nc.vector
                eng.tensor_tensor(out=s2[:, :], in0=h2t[m][:, :],
                                  in1=h2t[m][:, :], op=ALU.mult)
                h2sq.append(s2)

            # ------- gt = C^T d1 (m-outer, 2 gt banks) --------------------
            # E = (h2sq - 1) * gt = -d2*gt ; div = (-1)^T sum E
            ee = []
            for m in range(NK):
                gt = gps("gt")
                for k in range(NK):
                    nc.tensor.matmul(gt[:, :],
                                     cmk[k][:, m * 128:(m + 1) * 128],
                                     d1t[k][:, :],
                                     start=(k == 0), stop=(k == NK - 1))
                e = wpool.tile([128, BC], bf, name=f"e_{m}")
                nc.vector.scalar_tensor_tensor(out=e[:, :], in0=h2sq[m][:, :],
                                               scalar=1.0, in1=gt[:, :],
                                               op0=ALU.subtract, op1=ALU.mult)
                ee.append(e)

            # -------- dx = W3^T h2 (+b3 via copy-bias) --------------------
            outt = wpool.tile([D + 1, BC], f32, name="outt")
            dx_ps = small_ps("dx_ps", [D, BC])
            for k in range(NK):
                nc.tensor.matmul(dx_ps[:, :], w3p[k], h2t[k][:, :],
                                 start=(k == 0), stop=(k == NK - 1))
            nc.scalar.activation(outt[0:D, :], dx_ps[:, :], AF.Identity,
                                 bias=hot[0:D, 8:9])

            div_ps = small_ps("div_ps", [1, BC])
            for k in range(NK):
                nc.tensor.matmul(div_ps[:, :], neg_col, ee[k][:, :],
                                 start=(k == 0), stop=(k == NK - 1))
            nc.scalar.activation(outt[D:D + 1, :], div_ps[:, :], AF.Copy)

            # ------- store feature-major; host transposes -----------------
            nc.sync.dma_start(out=out_ext[:, :], in_=outt[:, :])

    nc.compile()
    return nc


def _get_nc():
    if "nc" not in _CACHE:
        _CACHE["nc"] = _build()
    return _CACHE["nc"]


def _prep_in_maps(t, x, W1, b1, W2, b2, W3, b3):
    t = np.asarray(t, dtype=np.float32)
    x = np.asarray(x, dtype=np.float32)
    W1 = np.asarray(W1, dtype=np.float32)
    b1 = np.asarray(b1, dtype=np.float32)
    W2 = np.asarray(W2, dtype=np.float32)
    W3 = np.asarray(W3, dtype=np.float32)

    xT = x[:, :D].T.astype(BF16)                       # (32, 2048)
    w1p = W1[:D].astype(BF16)                          # (32, 512)
    def chunkpack(M):
        return np.ascontiguousarray(
            M.reshape(NK, 128, H).transpose(1, 0, 2).reshape(128, NK * H))

    w2b = chunkpack(W2.astype(BF16))
    cm = chunkpack((W2 * (W3 @ W1[:D]).T).astype(BF16))

    w3pn = np.zeros((128, 129), dtype=BF16)
    w3pn[:, 0:128] = W3.reshape(NK, 128, D).transpose(1, 0, 2).reshape(128, 128).astype(BF16)
    w3pn[:, 128] = BF16(-1.0)

    hot = np.zeros((128, 9), dtype=np.float32)
    hot[:, 0:4] = np.asarray(b2, dtype=np.float32).reshape(NK, 128).T
    bias1 = (np.float32(t.ravel()[0]) * W1[D, :] + b1).astype(np.float32)
    hot[:, 4:8] = bias1.reshape(NK, 128).T
    hot[0:D, 8] = np.asarray(b3, dtype=np.float32)

    in_maps = []
    for i in range(NCORES):
        big0 = np.concatenate([xT[:, i * BC:(i + 1) * BC], w1p], axis=1)
        in_maps.append({
            "big0": np.ascontiguousarray(big0),
            "w2": w2b, "cm": cm, "w3pn": w3pn, "hot": hot,
        })
    return in_maps


def kernel(t, x, W1, b1, W2, b2, W3, b3):
    from concourse.bass_utils import run_bass_kernel_spmd

    nc = _get_nc()
    in_maps = _prep_in_maps(t, x, W1, b1, W2, b2, W3, b3)
    res = run_bass_kernel_spmd(nc, in_maps, core_ids=list(range(NCORES)))
    return np.concatenate(
        [np.ascontiguousarray(res.results[i]["out"].T) for i in range(NCORES)],
        axis=0)


# revision 16
# speedup vs baseline: 1.0635x; 1.0635x over previous
"""CNF vector-field + exact Jacobian-trace kernel for Trainium2 (8 NeuronCores).

Math: for each sample x (D=32), with inp = [x, t] (33,):
  h1 = tanh(inp @ W1 + b1); h2 = tanh(h1 @ W2 + b2); dx = h2 @ W3 + b3
  div = trace(J) = d1^T C d2,  C = W2 * (W3 @ W1r)^T  (elementwise),
  d_i = 1 - h_i^2,  W1r = W1[:32]
  out = [dx, div]  (B, 33)

v5 implementation notes:
  - all layout work on HOST: x^T, W3 chunk-packed, biases folded
    (bias1 = t*W1[32]+b1), and C itself precomputed on host (weight-only)
    so the device runs no mp/cmat chain at all
  - full bf16 datapath (PSUM accumulate stays f32), rel err ~5e-3 vs
    the 2e-2 gate
  - d1 = 1 - h1^2 (GpSimd square + DVE affine); h2sq split GpSimd/DVE;
    b3 via ACT Identity copy-bias; div via (-1)-column matmul over
    E = (h2sq-1)*gt
  - PSUM: one open accumulation group per bank (hw requirement):
    z tag 4 banks (z1 cycle into z2), gt tag 2 banks (m-outer), small 2
  - two HWDGE rings, 2-chunk splits: sync = hot,w2a,w2b + out;
    scalar = big0,cma,cmb,w3pn. z2 runs k-pair super-rounds as W2
    halves land; ACT table preload emitted after the scalar issues
  - PE warmup spinner ramps the DVFS clock during the DMA-wait window
"""
import sys

for _p in ("/opt/trn_rl_repo", "/root/.axon_site/_ro/trn_rl_repo"):
    if _p not in sys.path:
        sys.path.append(_p)

import numpy as np
import ml_dtypes

BF16 = ml_dtypes.bfloat16
B, D, H = 2048, 32, 512
NCORES = 8
BC = B // NCORES          # 256 rows per core
NK = H // 128             # 4 chunks of the hidden dim
WARMUP = 12               # PE clock-ramp spinner matmuls

_CACHE = {}


def _build():
    import concourse.bass as bass
    import concourse.tile as tile
    from concourse import bacc, mybir

    f32 = mybir.dt.float32
    bf = mybir.dt.bfloat16
    AF = mybir.ActivationFunctionType
    ALU = mybir.AluOpType

    nc = bacc.Bacc("TRN2", target_bir_lowering=False, debug=False,
                   num_devices=NCORES)

    # big0 cols: [0:256]=x^T slice, [256:768]=W1r
    big0_ext = nc.dram_tensor("big0", [D, BC + H], bf,
                              kind="ExternalInput").ap()
    # chunk-packed on host: w2p[p, k*512+j] = W2[k*128+p, j]
    w2_ext = nc.dram_tensor("w2", [128, NK * H], bf, kind="ExternalInput").ap()
    cm_ext = nc.dram_tensor("cm", [128, NK * H], bf, kind="ExternalInput").ap()
    # w3pn cols: [0:128]=W3 chunk-packed (lhsT for dx), 128=-1
    w3pn_ext = nc.dram_tensor("w3pn", [128, 129], bf,
                              kind="ExternalInput").ap()
    # hot cols: [0:4]=b2 col-major, [4:8]=bias1 col-major, 8=b3 (rows 0:32)
    hot_ext = nc.dram_tensor("hot", [128, 9], f32, kind="ExternalInput").ap()
    out_ext = nc.dram_tensor("out", [D + 1, BC], f32, kind="ExternalOutput").ap()

    with tile.TileContext(nc) as tc:
        with tc.tile_pool(name="const", bufs=1) as cpool, \
             tc.tile_pool(name="work", bufs=1) as wpool, \
             tc.tile_pool(name="ps", bufs=1, space="PSUM") as pps:

            # One open accumulation group per PSUM bank (hw requirement).
            def zps(nm):
                return pps.tile([128, BC], f32, name=nm, tag="z", bufs=4)

            def gps(nm):
                return pps.tile([128, BC], f32, name=nm, tag="gt", bufs=2)

            def small_ps(nm, shape):
                return pps.tile(shape, f32, name=nm, tag="small", bufs=2)

            # -------- PE warmup spinner (ramps clock during DMA wait) -----
            wt = wpool.tile([1, BC], bf, name="wt")
            nc.gpsimd.memset(wt[:, :], 0.0)
            warm = small_ps("warm", [1, BC])
            for _ in range(WARMUP):
                nc.tensor.matmul(warm[:, :], wt[:, 0:1], wt[:, :],
                                 start=True, stop=True)

            # ------------- input DMAs (two rings, need-ordered) -----------
            hot = cpool.tile([128, 9], f32, name="hot")
            nc.sync.dma_start(out=hot[:, :], in_=hot_ext[:, :])

            big0 = cpool.tile([D, BC + H], bf, name="big0")
            nc.scalar.dma_start(out=big0[:, :], in_=big0_ext[:, :])
            xts = big0[:, 0:BC]
            w1p = big0[:, BC:BC + H]

            w2all = cpool.tile([128, NK * H], bf, name="w2all")
            for k in range(NK):
                nc.sync.dma_start(out=w2all[:, k * H:(k + 1) * H],
                                  in_=w2_ext[:, k * H:(k + 1) * H])
            w2k = [w2all[:, k * H:(k + 1) * H] for k in range(NK)]

            cmat = cpool.tile([128, NK * H], bf, name="cmat")
            for k in range(NK):
                nc.scalar.dma_start(out=cmat[:, k * H:(k + 1) * H],
                                    in_=cm_ext[:, k * H:(k + 1) * H])
            cmk = [cmat[:, k * H:(k + 1) * H] for k in range(NK)]

            w3pn = cpool.tile([128, 129], bf, name="w3pn")
            nc.scalar.dma_start(out=w3pn[:, :], in_=w3pn_ext[:, :])
            w3p = [w3pn[:, k * D:(k + 1) * D] for k in range(NK)]
            neg_col = w3pn[:, 128:129]

            # -------- ACT spline-table preload (after scalar issues) ------
            dm0 = wpool.tile([1, 1], f32, name="dm0")
            dm1 = wpool.tile([1, 1], f32, name="dm1")
            nc.gpsimd.memset(dm0[:, :], 0.0)
            nc.scalar.activation(dm1[:, :], dm0[:, :], AF.Tanh)

            # ---------------- layer 1 matmuls, then all tanh --------------
            z1s = []
            for m in range(NK):
                z1 = zps("z1")
                nc.tensor.matmul(z1[:, :], w1p[:, m * 128:(m + 1) * 128],
                                 xts[:, :], start=True, stop=True)
                z1s.append(z1)
            h1t = []
            for m in range(NK):
                h = wpool.tile([128, BC], bf, name=f"h1t_{m}")
                nc.scalar.activation(h[:, :], z1s[m][:, :], AF.Tanh,
                                     bias=hot[:, 4 + m:5 + m])
                h1t.append(h)

            # ---------------- layer 2: k-pair super-rounds ----------------
            z2s = [zps("z2") for _ in range(NK)]
            for k in range(NK):
                for m in range(NK):
                    nc.tensor.matmul(z2s[m][:, :],
                                     w2k[k][:, m * 128:(m + 1) * 128],
                                     h1t[k][:, :],
                                     start=(k == 0), stop=(k == NK - 1))

            # ------- d1 = 1 - h1^2 (GpSimd sq, DVE affine) ----------------
            d1t = []
            for m in range(NK):
                sq = wpool.tile([128, BC], bf, name=f"sq_{m}")
                nc.gpsimd.tensor_tensor(out=sq[:, :], in0=h1t[m][:, :],
                                        in1=h1t[m][:, :], op=ALU.mult)
                d1 = wpool.tile([128, BC], bf, name=f"d1_{m}")
                nc.vector.tensor_scalar(out=d1[:, :], in0=sq[:, :],
                                        scalar1=-1.0, scalar2=1.0,
                                        op0=ALU.mult, op1=ALU.add)
                d1t.append(d1)

            h2t = []
            for m in range(NK):
                h = wpool.tile([128, BC], bf, name=f"h2t_{m}")
                nc.scalar.activation(h[:, :], z2s[m][:, :], AF.Tanh,
                                     bias=hot[:, m:1 + m])
                h2t.append(h)

            # -------- h2sq split across GpSimd and DVE --------------------
            h2sq = []
            for m in range(NK):
                s2 = wpool.tile([128, BC], bf, name=f"h2sq_{m}")
                eng = nc.gpsimd if m < 2 else nc.vector
                eng.tensor_tensor(out=s2[:, :], in0=h2t[m][:, :],
                                  in1=h2t[m][:, :], op=ALU.mult)
                h2sq.append(s2)

            # ------- gt = C^T d1 (m-outer, 2 gt banks) --------------------
            # E = (h2sq - 1) * gt = -d2*gt ; div = (-1)^T sum E
            outt = wpool.tile([D + 1, BC], f32, name="outt")
            ee = []

            def emit_gt(m):
                gt = gps("gt")
                for k in range(NK):
                    nc.tensor.matmul(gt[:, :],
                                     cmk[k][:, m * 128:(m + 1) * 128],
                                     d1t[k][:, :],
                                     start=(k == 0), stop=(k == NK - 1))
                e = wpool.tile([128, BC], bf, name=f"e_{m}")
                nc.vector.scalar_tensor_tensor(out=e[:, :], in0=h2sq[m][:, :],
                                               scalar=1.0, in1=gt[:, :],
                                               op0=ALU.subtract, op1=ALU.mult)
                ee.append(e)

            emit_gt(0)
            emit_gt(1)
            emit_gt(2)

            # -------- dx = W3^T h2 (+b3 via copy-bias), woven in ----------
            dx_ps = small_ps("dx_ps", [D, BC])
            for k in range(NK):
                nc.tensor.matmul(dx_ps[:, :], w3p[k], h2t[k][:, :],
                                 start=(k == 0), stop=(k == NK - 1))
            nc.scalar.activation(outt[0:D, :], dx_ps[:, :], AF.Identity,
                                 bias=hot[0:D, 8:9])

            emit_gt(3)

            div_ps = small_ps("div_ps", [1, BC])
            for k in range(NK):
                nc.tensor.matmul(div_ps[:, :], neg_col, ee[k][:, :],
                                 start=(k == 0), stop=(k == NK - 1))
            nc.scalar.activation(outt[D:D + 1, :], div_ps[:, :], AF.Copy)

            # ------- store feature-major; host transposes -----------------
            nc.sync.dma_start(out=out_ext[:, :], in_=outt[:, :])

    nc.compile()
    return nc


def _get_nc():
    if "nc" not in _CACHE:
        _CACHE["nc"] = _build()
    return _CACHE["nc"]


def _prep_in_maps(t, x, W1, b1, W2, b2, W3, b3):
    t = np.asarray(t, dtype=np.float32)
    x = np.asarray(x, dtype=np.float32)
    W1 = np.asarray(W1, dtype=np.float32)
    b1 = np.asarray(b1, dtype=np.float32)
    W2 = np.asarray(W2, dtype=np.float32)
    W3 = np.asarray(W3, dtype=np.float32)

    xT = x[:, :D].T.astype(BF16)                       # (32, 2048)
    w1p = W1[:D].astype(BF16)                          # (32, 512)
    def chunkpack(M):
        return np.ascontiguousarray(
            M.reshape(NK, 128, H).transpose(1, 0, 2).reshape(128, NK * H))

    w2b = chunkpack(W2.astype(BF16))
    cm = chunkpack((W2 * (W3 @ W1[:D]).T).astype(BF16))

    w3pn = np.zeros((128, 129), dtype=BF16)
    w3pn[:, 0:128] = W3.reshape(NK, 128, D).transpose(1, 0, 2).reshape(128, 128).astype(BF16)
    w3pn[:, 128] = BF16(-1.0)

    hot = np.zeros((128, 9), dtype=np.float32)
    hot[:, 0:4] = np.asarray(b2, dtype=np.float32).reshape(NK, 128).T
    bias1 = (np.float32(t.ravel()[0]) * W1[D, :] + b1).astype(np.float32)
    hot[:, 4:8] = bias1.reshape(NK, 128).T
    hot[0:D, 8] = np.asarray(b3, dtype=np.float32)

    in_maps = []
    for i in range(NCORES):
        big0 = np.concatenate([xT[:, i * BC:(i + 1) * BC], w1p], axis=1)
        in_maps.append({
            "big0": np.ascontiguousarray(big0),
            "w2": w2b, "cm": cm, "w3pn": w3pn, "hot": hot,
        })
    return in_maps


def kernel(t, x, W1, b1, W2, b2, W3, b3):
    from concourse.bass_utils import run_bass_kernel_spmd

    nc = _get_nc()
    in_maps = _prep_in_maps(t, x, W1, b1, W2, b2, W3, b3)
    res = run_bass_kernel_spmd(nc, in_maps, core_ids=list(range(NCORES)))
    return np.concatenate(
        [np.ascontiguousarray(res.results[i]["out"].T) for i in range(NCORES)],
        axis=0)


# revision 18
# speedup vs baseline: 1.1392x; 1.0712x over previous
"""CNF vector-field + exact Jacobian-trace kernel for Trainium2 (8 NeuronCores).

Math: for each sample x (D=32), with inp = [x, t] (33,):
  h1 = tanh(inp @ W1 + b1); h2 = tanh(h1 @ W2 + b2); dx = h2 @ W3 + b3
  div = trace(J) = d1^T C d2,  C = W2 * (W3 @ W1r)^T  (elementwise),
  d_i = 1 - h_i^2,  W1r = W1[:32]
  out = [dx, div]  (B, 33)

v5 implementation notes:
  - all layout work on HOST: x^T, W3 chunk-packed, biases folded
    (bias1 = t*W1[32]+b1), and C itself precomputed on host (weight-only)
    so the device runs no mp/cmat chain at all
  - full bf16 datapath (PSUM accumulate stays f32), rel err ~5e-3 vs
    the 2e-2 gate
  - d1 = 1 - h1^2 (GpSimd square + DVE affine); h2sq split GpSimd/DVE;
    b3 via ACT Identity copy-bias; div via (-1)-column matmul over
    E = (h2sq-1)*gt
  - PSUM: one open accumulation group per bank (hw requirement):
    z tag 4 banks (z1 cycle into z2), gt tag 2 banks (m-outer), small 2
  - two HWDGE rings, 2-chunk splits: sync = hot,w2a,w2b + out;
    scalar = big0,cma,cmb,w3pn. z2 runs k-pair super-rounds as W2
    halves land; ACT table preload emitted after the scalar issues
  - PE warmup spinner ramps the DVFS clock during the DMA-wait window
"""
import sys

for _p in ("/opt/trn_rl_repo", "/root/.axon_site/_ro/trn_rl_repo"):
    if _p not in sys.path:
        sys.path.append(_p)

import numpy as np
import ml_dtypes

BF16 = ml_dtypes.bfloat16
B, D, H = 2048, 32, 512
NCORES = 8
BC = B // NCORES          # 256 rows per core
NK = H // 128             # 4 chunks of the hidden dim
WARMUP = 12               # PE clock-ramp spinner matmuls

_CACHE = {}


def _build():
    import concourse.bass as bass
    import concourse.tile as tile
    from concourse import bacc, mybir

    f32 = mybir.dt.float32
    bf = mybir.dt.bfloat16
    AF = mybir.ActivationFunctionType
    ALU = mybir.AluOpType

    nc = bacc.Bacc("TRN2", target_bir_lowering=False, debug=False,
                   num_devices=NCORES)

    # big0 cols: [0:256]=x^T slice, [256:768]=W1r
    big0_ext = nc.dram_tensor("big0", [D, BC + H], bf,
                              kind="ExternalInput").ap()
    # chunk-packed on host: w2p[p, k*512+j] = W2[k*128+p, j]
    w2_ext = nc.dram_tensor("w2", [128, NK * H], bf, kind="ExternalInput").ap()
    cm_ext = nc.dram_tensor("cm", [128, NK * H], bf, kind="ExternalInput").ap()
    # w3pn cols: [0:128]=W3 chunk-packed (lhsT for dx), 128=-1
    w3pn_ext = nc.dram_tensor("w3pn", [128, 129], bf,
                              kind="ExternalInput").ap()
    # hot cols: [0:4]=b2 col-major, [4:8]=bias1 col-major, 8=b3 (rows 0:32)
    hot_ext = nc.dram_tensor("hot", [128, 9], f32, kind="ExternalInput").ap()
    out_ext = nc.dram_tensor("out", [D + 1, BC], f32, kind="ExternalOutput").ap()

    with tile.TileContext(nc) as tc:
        with tc.tile_pool(name="const", bufs=1) as cpool, \
             tc.tile_pool(name="work", bufs=1) as wpool, \
             tc.tile_pool(name="ps", bufs=1, space="PSUM") as pps:

            # One open accumulation group per PSUM bank (hw requirement).
            def zps(nm):
                return pps.tile([128, BC], f32, name=nm, tag="z", bufs=4)

            def gps(nm):
                return pps.tile([128, BC], f32, name=nm, tag="gt", bufs=2)

            def small_ps(nm, shape):
                return pps.tile(shape, f32, name=nm, tag="small", bufs=2)

            # -------- PE warmup spinner (ramps clock during DMA wait) -----
            wt = wpool.tile([1, BC], bf, name="wt")
            nc.gpsimd.memset(wt[:, :], 0.0)
            warm = small_ps("warm", [1, BC])
            for _ in range(WARMUP):
                nc.tensor.matmul(warm[:, :], wt[:, 0:1], wt[:, :],
                                 start=True, stop=True)

            # ------------- input DMAs (two rings, need-ordered) -----------
            hot = cpool.tile([128, 9], f32, name="hot")
            nc.sync.dma_start(out=hot[:, :], in_=hot_ext[:, :])

            big0 = cpool.tile([D, BC + H], bf, name="big0")
            nc.scalar.dma_start(out=big0[:, :], in_=big0_ext[:, :])
            xts = big0[:, 0:BC]
            w1p = big0[:, BC:BC + H]

            w2all = cpool.tile([128, NK * H], bf, name="w2all")
            for half in range(2):
                nc.sync.dma_start(
                    out=w2all[:, half * 2 * H:(half + 1) * 2 * H],
                    in_=w2_ext[:, half * 2 * H:(half + 1) * 2 * H])
            w2k = [w2all[:, k * H:(k + 1) * H] for k in range(NK)]

            # cm halves split across the rings so neither delays the other's
            # consumers; scalar's issue queue stays short so tanh1 isn't
            # stuck behind descriptor generation
            cmat = cpool.tile([128, NK * H], bf, name="cmat")
            nc.scalar.dma_start(out=cmat[:, 0:2 * H], in_=cm_ext[:, 0:2 * H])
            nc.sync.dma_start(out=cmat[:, 2 * H:4 * H],
                              in_=cm_ext[:, 2 * H:4 * H])
            cmk = [cmat[:, k * H:(k + 1) * H] for k in range(NK)]

            w3pn = cpool.tile([128, 129], bf, name="w3pn")
            nc.scalar.dma_start(out=w3pn[:, :], in_=w3pn_ext[:, :])
            w3p = [w3pn[:, k * D:(k + 1) * D] for k in range(NK)]
            neg_col = w3pn[:, 128:129]

            # -------- ACT spline-table preload (after scalar issues) ------
            dm0 = wpool.tile([1, 1], f32, name="dm0")
            dm1 = wpool.tile([1, 1], f32, name="dm1")
            nc.gpsimd.memset(dm0[:, :], 0.0)
            nc.scalar.activation(dm1[:, :], dm0[:, :], AF.Tanh)

            # ---------------- layer 1 matmuls, then all tanh --------------
            z1s = []
            for m in range(NK):
                z1 = zps("z1")
                nc.tensor.matmul(z1[:, :], w1p[:, m * 128:(m + 1) * 128],
                                 xts[:, :], start=True, stop=True)
                z1s.append(z1)
            h1t = []
            for m in range(NK):
                h = wpool.tile([128, BC], bf, name=f"h1t_{m}")
                nc.scalar.activation(h[:, :], z1s[m][:, :], AF.Tanh,
                                     bias=hot[:, 4 + m:5 + m])
                h1t.append(h)

            # ---------------- layer 2: k-pair super-rounds ----------------
            z2s = [zps("z2") for _ in range(NK)]
            for k in range(NK):
                for m in range(NK):
                    nc.tensor.matmul(z2s[m][:, :],
                                     w2k[k][:, m * 128:(m + 1) * 128],
                                     h1t[k][:, :],
                                     start=(k == 0), stop=(k == NK - 1))

            # ------- d1 = 1 - h1^2 (sq split GpSimd/DVE, DVE affine) ------
            d1t = []
            for m in range(NK):
                sq = wpool.tile([128, BC], bf, name=f"sq_{m}")
                eng = nc.gpsimd if m < 2 else nc.vector
                eng.tensor_tensor(out=sq[:, :], in0=h1t[m][:, :],
                                  in1=h1t[m][:, :], op=ALU.mult)
                d1 = wpool.tile([128, BC], bf, name=f"d1_{m}")
                nc.vector.tensor_scalar(out=d1[:, :], in0=sq[:, :],
                                        scalar1=-1.0, scalar2=1.0,
                                        op0=ALU.mult, op1=ALU.add)
                d1t.append(d1)

            h2t = []
            for m in range(NK):
                h = wpool.tile([128, BC], bf, name=f"h2t_{m}")
                nc.scalar.activation(h[:, :], z2s[m][:, :], AF.Tanh,
                                     bias=hot[:, m:1 + m])
                h2t.append(h)

            # -------- h2sq split across GpSimd and DVE --------------------
            h2sq = []
            for m in range(NK):
                s2 = wpool.tile([128, BC], bf, name=f"h2sq_{m}")
                eng = nc.gpsimd if m < 2 else nc.vector
                eng.tensor_tensor(out=s2[:, :], in0=h2t[m][:, :],
                                  in1=h2t[m][:, :], op=ALU.mult)
                h2sq.append(s2)

            # ------- gt = C^T d1 (m-outer, 2 gt banks) --------------------
            # E = (h2sq - 1) * gt = -d2*gt ; div = (-1)^T sum E
            outt = wpool.tile([D + 1, BC], f32, name="outt")
            ee = []

            def emit_gt(m):
                gt = gps("gt")
                for k in range(NK):
                    nc.tensor.matmul(gt[:, :],
                                     cmk[k][:, m * 128:(m + 1) * 128],
                                     d1t[k][:, :],
                                     start=(k == 0), stop=(k == NK - 1))
                e = wpool.tile([128, BC], bf, name=f"e_{m}")
                nc.vector.scalar_tensor_tensor(out=e[:, :], in0=h2sq[m][:, :],
                                               scalar=1.0, in1=gt[:, :],
                                               op0=ALU.subtract, op1=ALU.mult)
                ee.append(e)

            emit_gt(0)
            emit_gt(1)
            emit_gt(2)

            # -------- dx = W3^T h2 (+b3 via copy-bias), woven in ----------
            dx_ps = small_ps("dx_ps", [D, BC])
            for k in range(NK):
                nc.tensor.matmul(dx_ps[:, :], w3p[k], h2t[k][:, :],
                                 start=(k == 0), stop=(k == NK - 1))
            nc.scalar.activation(outt[0:D, :], dx_ps[:, :], AF.Identity,
                                 bias=hot[0:D, 8:9])

            emit_gt(3)

            div_ps = small_ps("div_ps", [1, BC])
            for k in range(NK):
                nc.tensor.matmul(div_ps[:, :], neg_col, ee[k][:, :],
                                 start=(k == 0), stop=(k == NK - 1))
            nc.scalar.activation(outt[D:D + 1, :], div_ps[:, :], AF.Copy)

            # ------- store feature-major; host transposes -----------------
            nc.sync.dma_start(out=out_ext[:, :], in_=outt[:, :])

    nc.compile()
    return nc


def _get_nc():
    if "nc" not in _CACHE:
        _CACHE["nc"] = _build()
    return _CACHE["nc"]


def _prep_in_maps(t, x, W1, b1, W2, b2, W3, b3):
    t = np.asarray(t, dtype=np.float32)
    x = np.asarray(x, dtype=np.float32)
    W1 = np.asarray(W1, dtype=np.float32)
    b1 = np.asarray(b1, dtype=np.float32)
    W2 = np.asarray(W2, dtype=np.float32)
    W3 = np.asarray(W3, dtype=np.float32)

    xT = x[:, :D].T.astype(BF16)                       # (32, 2048)
    w1p = W1[:D].astype(BF16)                          # (32, 512)
    def chunkpack(M):
        return np.ascontiguousarray(
            M.reshape(NK, 128, H).transpose(1, 0, 2).reshape(128, NK * H))

    w2b = chunkpack(W2.astype(BF16))
    cm = chunkpack((W2 * (W3 @ W1[:D]).T).astype(BF16))

    w3pn = np.zeros((128, 129), dtype=BF16)
    w3pn[:, 0:128] = W3.reshape(NK, 128, D).transpose(1, 0, 2).reshape(128, 128).astype(BF16)
    w3pn[:, 128] = BF16(-1.0)

    hot = np.zeros((128, 9), dtype=np.float32)
    hot[:, 0:4] = np.asarray(b2, dtype=np.float32).reshape(NK, 128).T
    bias1 = (np.float32(t.ravel()[0]) * W1[D, :] + b1).astype(np.float32)
    hot[:, 4:8] = bias1.reshape(NK, 128).T
    hot[0:D, 8] = np.asarray(b3, dtype=np.float32)

    in_maps = []
    for i in range(NCORES):
        big0 = np.concatenate([xT[:, i * BC:(i + 1) * BC], w1p], axis=1)
        in_maps.append({
            "big0": np.ascontiguousarray(big0),
            "w2": w2b, "cm": cm, "w3pn": w3pn, "hot": hot,
        })
    return in_maps


def kernel(t, x, W1, b1, W2, b2, W3, b3):
    from concourse.bass_utils import run_bass_kernel_spmd

    nc = _get_nc()
    in_maps = _prep_in_maps(t, x, W1, b1, W2, b2, W3, b3)
    res = run_bass_kernel_spmd(nc, in_maps, core_ids=list(range(NCORES)))
    return np.concatenate(
        [np.ascontiguousarray(res.results[i]["out"].T) for i in range(NCORES)],
        axis=0)


# revision 22
# speedup vs baseline: 1.1659x; 1.0235x over previous
"""CNF vector-field + exact Jacobian-trace kernel for Trainium2 (8 NeuronCores).

Math: for each sample x (D=32), with inp = [x, t] (33,):
  h1 = tanh(inp @ W1 + b1); h2 = tanh(h1 @ W2 + b2); dx = h2 @ W3 + b3
  div = trace(J) = d1^T C d2,  C = W2 * (W3 @ W1r)^T  (elementwise),
  d_i = 1 - h_i^2,  W1r = W1[:32]
  out = [dx, div]  (B, 33)

v5 implementation notes:
  - all layout work on HOST: x^T, W3 chunk-packed, biases folded
    (bias1 = t*W1[32]+b1), and C itself precomputed on host (weight-only)
    so the device runs no mp/cmat chain at all
  - full bf16 datapath (PSUM accumulate stays f32), rel err ~5e-3 vs
    the 2e-2 gate
  - d1 = 1 - h1^2 (GpSimd square + DVE affine); h2sq split GpSimd/DVE;
    b3 via ACT Identity copy-bias; div via (-1)-column matmul over
    E = (h2sq-1)*gt
  - PSUM: one open accumulation group per bank (hw requirement):
    z tag 4 banks (z1 cycle into z2), gt tag 2 banks (m-outer), small 2
  - two HWDGE rings, 2-chunk splits: sync = hot,w2a,w2b + out;
    scalar = big0,cma,cmb,w3pn. z2 runs k-pair super-rounds as W2
    halves land; ACT table preload emitted after the scalar issues
  - PE warmup spinner ramps the DVFS clock during the DMA-wait window
"""
import sys

for _p in ("/opt/trn_rl_repo", "/root/.axon_site/_ro/trn_rl_repo"):
    if _p not in sys.path:
        sys.path.append(_p)

import numpy as np
import ml_dtypes

BF16 = ml_dtypes.bfloat16
B, D, H = 2048, 32, 512
NCORES = 8
BC = B // NCORES          # 256 rows per core
NK = H // 128             # 4 chunks of the hidden dim
WARMUP = 12               # PE clock-ramp spinner matmuls

_CACHE = {}


def _build():
    import concourse.bass as bass
    import concourse.tile as tile
    from concourse import bacc, mybir

    f32 = mybir.dt.float32
    bf = mybir.dt.bfloat16
    AF = mybir.ActivationFunctionType
    ALU = mybir.AluOpType

    nc = bacc.Bacc("TRN2", target_bir_lowering=False, debug=False,
                   num_devices=NCORES)

    # big0 cols: [0:256]=x^T slice, [256:768]=W1r
    big0_ext = nc.dram_tensor("big0", [D, BC + H], bf,
                              kind="ExternalInput").ap()
    # chunk-packed on host: w2p[p, k*512+j] = W2[k*128+p, j]
    w2_ext = nc.dram_tensor("w2", [128, NK * H], bf, kind="ExternalInput").ap()
    cm_ext = nc.dram_tensor("cm", [128, NK * H], bf, kind="ExternalInput").ap()
    # w3pn cols: [0:128]=W3 chunk-packed (lhsT for dx), 128=-1
    w3pn_ext = nc.dram_tensor("w3pn", [128, 129], bf,
                              kind="ExternalInput").ap()
    # hot cols: [0:4]=b2 col-major, [4:8]=bias1 col-major, 8=b3 (rows 0:32)
    hot_ext = nc.dram_tensor("hot", [128, 9], f32, kind="ExternalInput").ap()
    out_ext = nc.dram_tensor("out", [D + 1, BC], bf, kind="ExternalOutput").ap()

    with tile.TileContext(nc) as tc:
        with tc.tile_pool(name="const", bufs=1) as cpool, \
             tc.tile_pool(name="work", bufs=1) as wpool, \
             tc.tile_pool(name="ps", bufs=1, space="PSUM") as pps:

            # One open accumulation group per PSUM bank (hw requirement).
            def zps(nm):
                return pps.tile([128, BC], f32, name=nm, tag="z", bufs=4)

            def gps(nm):
                return pps.tile([128, BC], f32, name=nm, tag="gt", bufs=2)

            def small_ps(nm, shape):
                return pps.tile(shape, f32, name=nm, tag="small", bufs=2)

            # -------- PE warmup spinner (ramps clock during DMA wait) -----
            wt = wpool.tile([1, BC], bf, name="wt")
            nc.gpsimd.memset(wt[:, :], 0.0)
            warm = small_ps("warm", [1, BC])
            for _ in range(WARMUP):
                nc.tensor.matmul(warm[:, :], wt[:, 0:1], wt[:, :],
                                 start=True, stop=True)

            # ------------- input DMAs (two rings, need-ordered) -----------
            hot = cpool.tile([128, 9], f32, name="hot")
            nc.sync.dma_start(out=hot[:, :], in_=hot_ext[:, :])

            big0 = cpool.tile([D, BC + H], bf, name="big0")
            nc.scalar.dma_start(out=big0[:, :], in_=big0_ext[:, :])
            xts = big0[:, 0:BC]
            w1p = big0[:, BC:BC + H]

            w2all = cpool.tile([128, NK * H], bf, name="w2all")
            for half in range(2):
                nc.sync.dma_start(
                    out=w2all[:, half * 2 * H:(half + 1) * 2 * H],
                    in_=w2_ext[:, half * 2 * H:(half + 1) * 2 * H])
            w2k = [w2all[:, k * H:(k + 1) * H] for k in range(NK)]

            # cm halves split across the rings so neither delays the other's
            # consumers; scalar's issue queue stays short so tanh1 isn't
            # stuck behind descriptor generation
            cmat = cpool.tile([128, NK * H], bf, name="cmat")
            nc.scalar.dma_start(out=cmat[:, 0:2 * H], in_=cm_ext[:, 0:2 * H])
            nc.sync.dma_start(out=cmat[:, 2 * H:4 * H],
                              in_=cm_ext[:, 2 * H:4 * H])
            cmk = [cmat[:, k * H:(k + 1) * H] for k in range(NK)]

            w3pn = cpool.tile([128, 129], bf, name="w3pn")
            nc.scalar.dma_start(out=w3pn[:, :], in_=w3pn_ext[:, :])
            w3p = [w3pn[:, k * D:(k + 1) * D] for k in range(NK)]
            neg_col = w3pn[:, 128:129]

            # -------- ACT spline-table preload (after scalar issues) ------
            dm0 = wpool.tile([1, 1], f32, name="dm0")
            dm1 = wpool.tile([1, 1], f32, name="dm1")
            nc.gpsimd.memset(dm0[:, :], 0.0)
            nc.scalar.activation(dm1[:, :], dm0[:, :], AF.Tanh)

            # ---------------- layer 1 matmuls, then all tanh --------------
            z1s = []
            for m in range(NK):
                z1 = zps("z1")
                nc.tensor.matmul(z1[:, :], w1p[:, m * 128:(m + 1) * 128],
                                 xts[:, :], start=True, stop=True)
                z1s.append(z1)
            h1t = []
            for m in range(NK):
                h = wpool.tile([128, BC], bf, name=f"h1t_{m}")
                nc.scalar.activation(h[:, :], z1s[m][:, :], AF.Tanh,
                                     bias=hot[:, 4 + m:5 + m])
                h1t.append(h)

            # ---------------- layer 2: k-pair super-rounds ----------------
            z2s = [zps("z2") for _ in range(NK)]
            for k in range(NK):
                for m in range(NK):
                    nc.tensor.matmul(z2s[m][:, :],
                                     w2k[k][:, m * 128:(m + 1) * 128],
                                     h1t[k][:, :],
                                     start=(k == 0), stop=(k == NK - 1))

            # ------- d1 = 1 - h1^2 (sq split GpSimd/DVE, DVE affine) ------
            d1t = []
            for m in range(NK):
                sq = wpool.tile([128, BC], bf, name=f"sq_{m}")
                eng = nc.gpsimd if m < 2 else nc.vector
                eng.tensor_tensor(out=sq[:, :], in0=h1t[m][:, :],
                                  in1=h1t[m][:, :], op=ALU.mult)
                d1 = wpool.tile([128, BC], bf, name=f"d1_{m}")
                nc.vector.tensor_scalar(out=d1[:, :], in0=sq[:, :],
                                        scalar1=-1.0, scalar2=1.0,
                                        op0=ALU.mult, op1=ALU.add)
                d1t.append(d1)

            h2t = []
            for m in range(NK):
                h = wpool.tile([128, BC], bf, name=f"h2t_{m}")
                nc.scalar.activation(h[:, :], z2s[m][:, :], AF.Tanh,
                                     bias=hot[:, m:1 + m])
                h2t.append(h)

            # -------- h2sq split across GpSimd and DVE --------------------
            h2sq = []
            for m in range(NK):
                s2 = wpool.tile([128, BC], bf, name=f"h2sq_{m}")
                eng = nc.gpsimd if m < 2 else nc.vector
                eng.tensor_tensor(out=s2[:, :], in0=h2t[m][:, :],
                                  in1=h2t[m][:, :], op=ALU.mult)
                h2sq.append(s2)

            # ------- gt = C^T d1 (m-outer, 2 gt banks) --------------------
            # E = (h2sq - 1) * gt = -d2*gt ; div = (-1)^T sum E
            outt = wpool.tile([D + 1, BC], bf, name="outt")
            ee = []

            def emit_gt(m):
                gt = gps("gt")
                for k in range(NK):
                    nc.tensor.matmul(gt[:, :],
                                     cmk[k][:, m * 128:(m + 1) * 128],
                                     d1t[k][:, :],
                                     start=(k == 0), stop=(k == NK - 1))
                e = wpool.tile([128, BC], bf, name=f"e_{m}")
                nc.vector.scalar_tensor_tensor(out=e[:, :], in0=h2sq[m][:, :],
                                               scalar=1.0, in1=gt[:, :],
                                               op0=ALU.subtract, op1=ALU.mult)
                ee.append(e)

            emit_gt(0)
            emit_gt(1)
            emit_gt(2)

            # -------- dx = W3^T h2 (+b3 via copy-bias), woven in ----------
            dx_ps = small_ps("dx_ps", [D, BC])
            for k in range(NK):
                nc.tensor.matmul(dx_ps[:, :], w3p[k], h2t[k][:, :],
                                 start=(k == 0), stop=(k == NK - 1))
            nc.scalar.activation(outt[0:D, :], dx_ps[:, :], AF.Identity,
                                 bias=hot[0:D, 8:9])

            emit_gt(3)

            div_ps = small_ps("div_ps", [1, BC])
            for k in range(NK):
                nc.tensor.matmul(div_ps[:, :], neg_col, ee[k][:, :],
                                 start=(k == 0), stop=(k == NK - 1))
            nc.vector.tensor_copy(outt[D:D + 1, :], div_ps[:, :])

            # ------- store feature-major; host transposes -----------------
            nc.sync.dma_start(out=out_ext[:, :], in_=outt[:, :])

    nc.compile()
    return nc


def _get_nc():
    if "nc" not in _CACHE:
        _CACHE["nc"] = _build()
    return _CACHE["nc"]


def _prep_in_maps(t, x, W1, b1, W2, b2, W3, b3):
    t = np.asarray(t, dtype=np.float32)
    x = np.asarray(x, dtype=np.float32)
    W1 = np.asarray(W1, dtype=np.float32)
    b1 = np.asarray(b1, dtype=np.float32)
    W2 = np.asarray(W2, dtype=np.float32)
    W3 = np.asarray(W3, dtype=np.float32)

    xT = x[:, :D].T.astype(BF16)                       # (32, 2048)
    w1p = W1[:D].astype(BF16)                          # (32, 512)
    def chunkpack(M):
        return np.ascontiguousarray(
            M.reshape(NK, 128, H).transpose(1, 0, 2).reshape(128, NK * H))

    w2b = chunkpack(W2.astype(BF16))
    cm = chunkpack((W2 * (W3 @ W1[:D]).T).astype(BF16))

    w3pn = np.zeros((128, 129), dtype=BF16)
    w3pn[:, 0:128] = W3.reshape(NK, 128, D).transpose(1, 0, 2).reshape(128, 128).astype(BF16)
    w3pn[:, 128] = BF16(-1.0)

    hot = np.zeros((128, 9), dtype=np.float32)
    hot[:, 0:4] = np.asarray(b2, dtype=np.float32).reshape(NK, 128).T
    bias1 = (np.float32(t.ravel()[0]) * W1[D, :] + b1).astype(np.float32)
    hot[:, 4:8] = bias1.reshape(NK, 128).T
    hot[0:D, 8] = np.asarray(b3, dtype=np.float32)

    in_maps = []
    for i in range(NCORES):
        big0 = np.concatenate([xT[:, i * BC:(i + 1) * BC], w1p], axis=1)
        in_maps.append({
            "big0": np.ascontiguousarray(big0),
            "w2": w2b, "cm": cm, "w3pn": w3pn, "hot": hot,
        })
    return in_maps


def kernel(t, x, W1, b1, W2, b2, W3, b3):
    from concourse.bass_utils import run_bass_kernel_spmd

    nc = _get_nc()
    in_maps = _prep_in_maps(t, x, W1, b1, W2, b2, W3, b3)
    res = run_bass_kernel_spmd(nc, in_maps, core_ids=list(range(NCORES)))
    return np.concatenate(
        [res.results[i]["out"].T.astype(np.float32) for i in range(NCORES)],
        axis=0)
